# revision 19
# baseline (speedup 1.0000x reference)
"""Multi-head attention on 8 NeuronCores (Trainium2, Bass/Tile).

Problem: B=2, S=2048, E=1024, H=16, D=64 MHA with int mask, fp32.

Sharding (per the tensor-parallel hint): core c = 4*b + g handles batch b,
head group g (4 heads = a 256-wide slice of E).  Q/K/V projections, scores,
softmax and attention are head-parallel; Wo is row-sharded so each core
emits a partial [S, E] output projection; the host sums the 4 partials per
batch (the all-reduce) and adds bo.

Device pipeline per core (S=2048, local j = h*64+d in [0,256)):
  qhT, khT : [j, S] fp32r (pair-major [128, pair, S]); produced by PE from
             bf16 x-transposed streams and bf16 weights, evacuated by ACT.
  vh       : [S, j] as [128, s_tile, head, 65] bf16 with a ones column ->
             P @ [vh|1] yields the softmax denominator for free.
  scores_T : [ks, q] fp32r matmuls into PSUM (K=64; head pairs land on
             different PE row groups so they overlap on HW); ACT exp
             (scale=1/8) -> bf16; DVE mask multiply (bf16 2x mode);
             PE accumulates ctx_T over ks.
  ctx_T    : normalized via reciprocal_approx_fast + partition_broadcast,
             stored [j, S] fp32r; partial out = ctx_T.T @ WoT on PE.
"""

import os
import sys

sys.path.insert(0, "/opt/trn_rl_repo")

import numpy as np
import ml_dtypes

import concourse.bass as bass
import concourse.mybir as mybir
import concourse.tile as tile
from concourse import bacc
from concourse import bass_utils

B, S, E, H = 2, 2048, 1024, 16
D = E // H              # 64
G = 4                   # head groups (cores per batch)
HL = H // G             # 4 local heads per core
J = HL * D              # 256 local j width
P = 128
KT = E // P             # 8 k-tiles for projections
ST = S // P             # 16 s-tiles / ks-tiles
NQ = 1024               # q-chunk width for attention
QC = S // NQ            # 2 q chunks
MC = 4                  # mask ks-tiles per DMA chunk

F32 = mybir.dt.float32
F32R = mybir.dt.float32r
F16 = mybir.dt.float16
BF16 = mybir.dt.bfloat16
F16 = mybir.dt.float16

# Exposed for test.py / bench.py.
LAST_RESULTS = None
LAST_NC = None


def _round_f32r(x: np.ndarray) -> np.ndarray:
    """Round fp32 to fp32r (tf32-like, keep 10 mantissa bits), RNE."""
    u = np.ascontiguousarray(x, dtype=np.float32).view(np.uint32)
    u = (u + 0x00000FFF + ((u >> 13) & 1)) & 0xFFFFE000
    return u.astype(np.uint32).view(np.float32)


def _bf16(x: np.ndarray) -> np.ndarray:
    return np.ascontiguousarray(x, dtype=np.float32).astype(np.float16)


def _build_program(use_bias_qk: bool, use_bias_v: bool):
    nc = bacc.Bacc("TRN2", target_bir_lowering=False, debug=False, num_devices=8)

    xqT = nc.dram_tensor("xqT", [E, S], F16, kind="ExternalInput")
    xkT = nc.dram_tensor("xkT", [E, S], F16, kind="ExternalInput")
    xvT = nc.dram_tensor("xvT", [E, S], F16, kind="ExternalInput")
    maskT = nc.dram_tensor("maskT", [S, S], F16, kind="ExternalInput")
    wqT = nc.dram_tensor("wqT", [E, J], F16, kind="ExternalInput")
    wkT = nc.dram_tensor("wkT", [E, J], F16, kind="ExternalInput")
    wvT = nc.dram_tensor("wvT", [E, J], F16, kind="ExternalInput")
    woT = nc.dram_tensor("woT", [J, E], F32R, kind="ExternalInput")
    bq = nc.dram_tensor("bq", [J], F32, kind="ExternalInput")
    bk = nc.dram_tensor("bk", [J], F32, kind="ExternalInput")
    bv = nc.dram_tensor("bv", [J], F32, kind="ExternalInput")
    out = nc.dram_tensor("out", [S, E], F16, kind="ExternalOutput")

    Copy = mybir.ActivationFunctionType.Copy
    Exp = mybir.ActivationFunctionType.Exp

    with tile.TileContext(nc) as tc:
        with (
            tc.tile_pool(name="consts", bufs=1) as consts,
            tc.tile_pool(name="persist", bufs=1) as persist,
            tc.tile_pool(name="xs", bufs=5) as xs,
            tc.tile_pool(name="xv", bufs=1) as xvpool,
            tc.tile_pool(name="maskp", bufs=4) as maskp,
            tc.tile_pool(name="pwork", bufs=6) as pwork,
            tc.tile_pool(name="osb", bufs=4) as osb,
            tc.tile_pool(name="small", bufs=2) as small,
        ):
            # ---- weights / constants ----
            wq_sb = consts.tile([P, KT, J], F16, tag="wq")
            wk_sb = consts.tile([P, KT, J], F16, tag="wk")
            wv_sb = consts.tile([P, KT, J], F16, tag="wv")
            wo_sb = consts.tile([P, J // P, E], F32R, tag="wo")

            if use_bias_qk:
                bq_sb = consts.tile([P, J // P], F32, tag="bq")
                bk_sb = consts.tile([P, J // P], F32, tag="bk")
                nc.sync.dma_start(bq_sb[:], bq.rearrange("(pr p) -> p pr", p=P))
                nc.sync.dma_start(bk_sb[:], bk.rearrange("(pr p) -> p pr", p=P))
            if use_bias_v:
                bv_row = consts.tile([1, J], F32, tag="bvr")
                nc.sync.dma_start(bv_row[:], bv.rearrange("j -> 1 j"))
                bv_bc = consts.tile([P, J], F32, tag="bvb")
                nc.gpsimd.partition_broadcast(bv_bc[:], bv_row[:])

            # ---- persistent activations ----
            qhT = persist.tile([P, 2, S], F32R, tag="qhT")
            khT = persist.tile([P, 2, S], F32R, tag="khT")
            vh = persist.tile([P, ST, HL, 65], F16, tag="vh")
            ctxT = persist.tile([P, 2, S], F32R, tag="ctxT")

            nc.gpsimd.memset(vh[:, :, :, 64:65], 1.0)

            # ---- phase A: projections ----
            projacc_cm = tc.tile_pool(name="projacc", bufs=8, space="PSUM")
            projacc = projacc_cm.__enter__()

            nc.sync.dma_start(wq_sb[:], wqT.rearrange("(kt p) j -> p kt j", p=P))
            nc.sync.dma_start(wk_sb[:], wkT.rearrange("(kt p) j -> p kt j", p=P))

            # q and k -> transposed layout [j, s], pair-major
            for w_sb, x_dram, outT, b_sb in (
                (wq_sb, xqT, qhT, "bq"),
                (wk_sb, xkT, khT, "bk"),
            ):
                accs = [projacc.tile([P, 512], F32, tag="pacc", name=f"pacc{i}")
                        for i in range(8)]
                for kt in range(KT):
                    xt = xs.tile([P, S], F16, tag="xt")
                    nc.sync.dma_start(xt[:], x_dram[kt * P:(kt + 1) * P, :])
                    for pair in range(2):
                        for n4 in range(4):
                            nc.tensor.matmul(
                                accs[pair * 4 + n4][:],
                                w_sb[:, kt, pair * P:(pair + 1) * P],
                                xt[:, n4 * 512:(n4 + 1) * 512],
                                start=(kt == 0), stop=(kt == KT - 1),
                            )
                for pair in range(2):
                    for n4 in range(4):
                        dst = outT[:, pair, n4 * 512:(n4 + 1) * 512]
                        src = accs[pair * 4 + n4][:]
                        if use_bias_qk:
                            bias = (bq_sb if b_sb == "bq" else bk_sb)[:, pair:pair + 1]
                            nc.scalar.activation(dst, src, Copy, bias=bias)
                        else:
                            nc.vector.tensor_copy(dst, src)
            # v -> natural layout [s, j]; x_v tiles stay resident, two psum
            # half-passes of 8 s-tiles (one bank per accumulator).
            nc.sync.dma_start(wv_sb[:], wvT.rearrange("(kt p) j -> p kt j", p=P))
            xvt = [xvpool.tile([P, S], F16, tag=f"xv{i}", name=f"xv{i}")
                   for i in range(KT)]
            for kt in range(KT):
                nc.sync.dma_start(xvt[kt][:], xvT[kt * P:(kt + 1) * P, :])
            for sh in range(2):
                vaccs = [projacc.tile([P, J], F32, tag="pacc", name=f"vacc{sh}_{i}")
                         for i in range(8)]
                for kt in range(KT):
                    for si in range(8):
                        st = sh * 8 + si
                        nc.tensor.matmul(
                            vaccs[si][:],
                            xvt[kt][:, st * P:(st + 1) * P],
                            wv_sb[:, kt, :],
                            start=(kt == 0), stop=(kt == KT - 1),
                        )
                for si in range(8):
                    st = sh * 8 + si
                    src3 = vaccs[si][:].rearrange("p (h d) -> p h d", h=HL)
                    dst = vh[:, st, :, 0:64]
                    if use_bias_v:
                        nc.vector.tensor_add(
                            dst, src3, bv_bc[:].rearrange("p (h d) -> p h d", h=HL)
                        )
                    else:
                        nc.vector.tensor_copy(dst, src3)

            projacc_cm.__exit__(None, None, None)

            # ---- phase B: attention ----
            stps_cm = tc.tile_pool(name="stps", bufs=2, space="PSUM")
            stps = stps_cm.__enter__()
            ctxps_cm = tc.tile_pool(name="ctxps", bufs=2, space="PSUM")
            ctxps = ctxps_cm.__enter__()
            for qc in range(QC):
                mtiles = {}
                for hp in range(2):          # head pair
                    cps = [ctxps.tile([65, NQ], F32, tag="cacc", name=f"cacc{i}")
                           for i in range(2)]
                    for ks in range(ST):
                        ci = ks // MC
                        if hp == 0 and ks % MC == 0:
                            mch = maskp.tile([P, MC, NQ], F16, tag="mch",
                                             name=f"mch{qc}_{ci}")
                            nc.sync.dma_start(
                                mch[:],
                                maskT[ks * P:(ks + MC) * P,
                                      qc * NQ:(qc + 1) * NQ].rearrange(
                                    "(kt p) q -> p kt q", p=P),
                            )
                            mtiles[ci] = mch
                        mcur = mtiles[ci]
                        for hh in range(2):  # head within pair -> PE row group
                            h = 2 * hp + hh
                            st_ = stps.tile([P, NQ], F32, tag="st")
                            for n2 in range(2):
                                nc.tensor.matmul(
                                    st_[:, n2 * 512:(n2 + 1) * 512],
                                    khT[hh * 64:(hh + 1) * 64, hp,
                                        ks * P:(ks + 1) * P],
                                    qhT[hh * 64:(hh + 1) * 64, hp,
                                        qc * NQ + n2 * 512:qc * NQ + (n2 + 1) * 512],
                                    start=True, stop=True,
                                )
                            p_t = pwork.tile([P, NQ], F16, tag="pt")
                            nc.scalar.activation(p_t[:], st_[:], Exp, scale=0.125)
                            nc.vector.tensor_mul(p_t[:], p_t[:],
                                                 mcur[:, ks % MC, :])
                            for n2 in range(2):
                                nc.tensor.matmul(
                                    cps[hh][:, n2 * 512:(n2 + 1) * 512],
                                    vh[:, ks, h, :],
                                    p_t[:, n2 * 512:(n2 + 1) * 512],
                                    start=(ks == 0), stop=(ks == ST - 1),
                                )
                    for hh in range(2):
                        rr = small.tile([1, NQ], F32, tag="rr", bufs=1)
                        nc.vector.reciprocal(rr[:], cps[hh][64:65, :])
                        rb = small.tile([64, NQ], F32, tag="rb")
                        nc.gpsimd.partition_broadcast(rb[:], rr[:])
                        nc.vector.tensor_mul(
                            ctxT[hh * 64:(hh + 1) * 64, hp,
                                 qc * NQ:(qc + 1) * NQ],
                            cps[hh][0:64, :],
                            rb[:],
                        )
            ctxps_cm.__exit__(None, None, None)
            stps_cm.__exit__(None, None, None)

            # ---- phase C: output projection (partial) ----
            nc.sync.dma_start(wo_sb[:], woT.rearrange("(kt p) e -> p kt e", p=P))
            outps_cm = tc.tile_pool(name="outps", bufs=2, space="PSUM")
            outps = outps_cm.__enter__()
            for st in range(ST):
                ops = outps.tile([P, NQ], F32, tag="ops")
                for ec in range(2):
                    for kt2 in range(2):
                        nc.tensor.matmul(
                            ops[:, ec * 512:(ec + 1) * 512],
                            ctxT[:, kt2, st * P:(st + 1) * P],
                            wo_sb[:, kt2, ec * 512:(ec + 1) * 512],
                            start=(kt2 == 0), stop=(kt2 == 1),
                        )
                o_sb = osb.tile([P, E], F16, tag="o")
                nc.scalar.activation(o_sb[:, 0:512], ops[:, 0:512], Copy)
                nc.vector.tensor_copy(o_sb[:, 512:1024], ops[:, 512:1024])
                nc.sync.dma_start(out[st * P:(st + 1) * P, :], o_sb[:])
            outps_cm.__exit__(None, None, None)

    nc.compile()
    return nc


def kernel(q, k, v, mask, Wq, bq, Wk, bk, Wv, bv, Wo, bo):
    global LAST_RESULTS
    q = np.asarray(q, np.float32)
    k = np.asarray(k, np.float32)
    v = np.asarray(v, np.float32)
    mask = np.asarray(mask)
    Wq = np.asarray(Wq, np.float32)
    Wk = np.asarray(Wk, np.float32)
    Wv = np.asarray(Wv, np.float32)
    Wo = np.asarray(Wo, np.float32)
    bq = np.asarray(bq, np.float32)
    bk = np.asarray(bk, np.float32)
    bv = np.asarray(bv, np.float32)
    bo = np.asarray(bo, np.float32)

    use_bias_qk = bool(np.any(bq) or np.any(bk))
    use_bias_v = bool(np.any(bv))

    global LAST_NC
    nc = _build_program(use_bias_qk, use_bias_v)
    LAST_NC = nc

    xT = {}
    for b in range(B):
        xT[("q", b)] = _bf16(q[b].T)
        xT[("k", b)] = _bf16(k[b].T)
        xT[("v", b)] = _bf16(v[b].T)
        xT[("m", b)] = _bf16(mask[b, 0].T.astype(np.float32))

    in_maps = []
    for c in range(8):
        b, g = divmod(c, G)
        js = slice(g * J, (g + 1) * J)
        in_maps.append({
            "xqT": xT[("q", b)],
            "xkT": xT[("k", b)],
            "xvT": xT[("v", b)],
            "maskT": xT[("m", b)],
            "wqT": _bf16(Wq[js, :].T),
            "wkT": _bf16(Wk[js, :].T),
            "wvT": _bf16(Wv[js, :].T),
            "woT": _round_f32r(Wo[:, js].T),
            "bq": np.ascontiguousarray(bq[js]),
            "bk": np.ascontiguousarray(bk[js]),
            "bv": np.ascontiguousarray(bv[js]),
        })

    os.environ["BASS_NEVER_TRACE"] = "1"
    res = bass_utils.run_bass_kernel_spmd(
        nc, in_maps, core_ids=list(range(8)), trace=False,
    )
    LAST_RESULTS = res

    full = np.zeros((B, S, E), np.float32)
    for c in range(8):
        b = c // G
        full[b] += res.results[c]["out"].astype(np.float32)
    full += bo[None, None, :]
    return full
